# revision 2
# baseline (speedup 1.0000x reference)
"""ChannelBlockImportanceGate kernel for 8 Trainium2 NeuronCores.

Computes, per (b, c) slice of features [8, 256, 132, 132] f32:
  scores = block-sum of |x| over 8x8 blocks (17x17 grid, zero-padded edges)
  top-72 blocks (ties -> lowest index, matching jax.lax.top_k)
  output = per-pixel {0,1} mask upsampled 8x8 (cropped to 132x132)

The straight-through soft term of the reference cancels in the forward
pass (hard - sg(soft) + soft == hard up to ~1ulp), so the output is the
hard mask.

Sharding: purely data parallel. 2048 (b,c) slices -> 256 per core.
Per core: 2 groups of 128 slices; each slice occupies one SBUF
partition so pooling/topk are per-partition ops with no cross-partition
traffic. Top-72 uses 9 rounds of DVE max8 + match_replace(-1e30), then
mask = (score < 0).

The 8x8 upsample is done BY THE STORE DMAs, not by compute engines: a
row-mask [128, 17*132] holds one 132-px row per block-row, and each
store DMA writes that 528B line 8x (stride-0 source broadcast) into the
8 identical output rows of the block-row. 528B descriptor lines are
>= the 512B full-bandwidth DMA cutoff, so stores stay at peak HBM
bandwidth while reading only ~9KB/partition from SBUF. This removes
the 40us scalar-engine upsample of v1 and lets stores begin as soon as
the topk mask is ready, overlapping the other group's loads.
"""

import numpy as np

B, C, H, W = 8, 256, 132, 132
HW = H * W            # 17424
NB = 17               # 8x8 blocks per side (132 padded to 136)
NBLK = NB * NB        # 289
KEEP = 72             # round(289 * 0.25)
N_CORES = 8
S = (B * C) // N_CORES  # 256 slices per core
ROW_CHUNKS = ((0, 32), (32, 64), (64, 96), (96, 132))
NEG = -1.0e30

_prog_cache = {}


def _build_program():
    import concourse.bacc as bacc
    import concourse.mybir as mybir
    import concourse.tile as tile

    f32 = mybir.dt.float32
    X = mybir.AxisListType.X
    ADD = mybir.AluOpType.add

    nc = bacc.Bacc("TRN2", debug=False, num_devices=N_CORES)
    x = nc.dram_tensor("x", (S, HW), f32, kind="ExternalInput")
    y = nc.dram_tensor("y", (S, HW), f32, kind="ExternalOutput")

    with tile.TileContext(nc) as tc:
        with (
            tc.tile_pool(name="big", bufs=2) as bigp,
            tc.tile_pool(name="med", bufs=2) as medp,
            tc.tile_pool(name="small", bufs=2) as smallp,
        ):
            for g in range(S // 128):
                p0 = g * 128
                chunks = []
                for k, (r0, r1) in enumerate(ROW_CHUNKS):
                    ch = bigp.tile([128, (r1 - r0) * W], f32,
                                   name=f"ch_g{g}k{k}", tag=f"chunk{k}")
                    nc.sync.dma_start(out=ch[:, :],
                                      in_=x[p0:p0 + 128, r0 * W:r1 * W])
                    chunks.append(ch)

                # W-pool: per image row, |x| summed over 8-col groups
                # (16 full groups + one 4-col partial group).
                wsum = medp.tile([128, H * NB], f32,
                                 name=f"wsum_g{g}", tag="wsum")
                ws3 = wsum.rearrange("p (r t) -> p r t", t=NB)
                for k, (r0, r1) in enumerate(ROW_CHUNKS):
                    v = chunks[k].rearrange("p (r w) -> p r w", w=W)
                    nc.vector.tensor_reduce(
                        out=ws3[:, r0:r1, 0:16],
                        in_=v[:, :, 0:128].rearrange("p r (q c) -> p r q c", c=8),
                        axis=X, op=ADD, apply_absolute_value=True)
                    nc.vector.tensor_reduce(
                        out=ws3[:, r0:r1, 16:17],
                        in_=v[:, :, 128:132],
                        axis=X, op=ADD, apply_absolute_value=True)

                # H-pool: row sums summed over 8-row groups (16 full + 4-row
                # partial) -> scores [128, 289], layout h*17 + w.
                scores = smallp.tile([128, NBLK], f32,
                                     name=f"scores_g{g}", tag="scores")
                sc3 = scores.rearrange("p (h t) -> p h t", t=NB)
                nc.vector.tensor_reduce(
                    out=sc3[:, 0:16, :],
                    in_=ws3[:, 0:128, :].rearrange("p (h r) t -> p h t r", r=8),
                    axis=X, op=ADD)
                nc.vector.tensor_reduce(
                    out=sc3[:, 16:17, :],
                    in_=ws3[:, 128:132, :].rearrange("p r t -> p t r"),
                    axis=X, op=ADD)

                # Top-72 per partition: 9 rounds of max8 + match_replace.
                # match_replace replaces the first unmatched occurrence, so
                # ties resolve to the lowest index like jax.lax.top_k.
                for it in range(KEEP // 8):
                    m8 = smallp.tile([128, 8], f32,
                                     name=f"m8_g{g}i{it}", tag="m8")
                    nc.vector.max(out=m8[:, :], in_=scores[:, :])
                    nc.vector.match_replace(out=scores[:, :],
                                            in_to_replace=m8[:, :],
                                            in_values=scores[:, :],
                                            imm_value=NEG)

                # Block mask: replaced entries are -1e30, real scores are >= 0.
                mask = smallp.tile([128, NBLK], f32,
                                   name=f"mask_g{g}", tag="mask")
                nc.vector.tensor_scalar(out=mask[:, :], in0=scores[:, :],
                                        scalar1=0.0, scalar2=None,
                                        op0=mybir.AluOpType.is_lt)
                m3 = mask.rearrange("p (h t) -> p h t", t=NB)

                # Row-mask [p, h, w]: one 132-px row per block-row, the
                # 8x horizontal upsample of the block mask.
                rm = medp.tile([128, NB * W], f32, name=f"rm_g{g}", tag="rm")
                rm3 = rm.rearrange("p (h w) -> p h w", w=W)
                nc.vector.tensor_copy(
                    out=rm3[:, :, 0:128].rearrange("p h (q c) -> p h q c", c=8),
                    in_=(m3[:, :, 0:16].unsqueeze(3)
                         .broadcast_to((128, NB, 16, 8))))
                nc.vector.tensor_copy(
                    out=rm3[:, :, 128:132],
                    in_=m3[:, :, 16:17].broadcast_to((128, NB, 4)))

                # Stores: the vertical 8x upsample happens inside the DMA via
                # a stride-0 source dim -- each 528B row-mask line is written
                # 8x (4x for the partial last block-row) to contiguous HBM.
                # Issued on the (otherwise idle) scalar engine's HWDGE queue
                # so store triggers never queue behind load triggers.
                for h in range(16):
                    nc.scalar.dma_start(
                        out=y[p0:p0 + 128, h * 8 * W:(h + 1) * 8 * W],
                        in_=rm3[:, h:h + 1, :].broadcast_to((128, 8, W)))
                nc.scalar.dma_start(
                    out=y[p0:p0 + 128, 128 * W:132 * W],
                    in_=rm3[:, 16:17, :].broadcast_to((128, 4, W)))
    nc.compile()
    return nc


def _ensure_ntff_hook_module():
    """bass_utils' trace path does `from antenv.axon_hooks import
    get_axon_ntff_profile_hook` — a module this image doesn't ship.
    Register an equivalent (ctypes into libaxon_pjrt.so, mirroring
    trn_boot._ntff_profile_via_ctypes) so BASS_TRACE=1 works; degrade
    to a None hook (trace skipped) when unavailable."""
    import sys
    import types

    try:
        import antenv.axon_hooks  # noqa: F401
        return
    except Exception:
        pass

    hook = None
    try:
        import contextlib
        import ctypes

        so_path = "/opt/axon/libaxon_pjrt.so"
        lib = ctypes.CDLL(so_path)
        if hasattr(lib, "axon_start_nrt_profile"):
            lib.axon_start_nrt_profile.argtypes = [
                ctypes.POINTER(ctypes.c_int64), ctypes.c_size_t]
            lib.axon_start_nrt_profile.restype = ctypes.c_int64
            lib.axon_stop_nrt_profile.argtypes = [ctypes.c_char_p]
            lib.axon_stop_nrt_profile.restype = ctypes.c_int64

            @contextlib.contextmanager
            def _hook(output_dir, device_ids):
                import jax
                jax.devices()
                if device_ids:
                    ids = (ctypes.c_int64 * len(device_ids))(*device_ids)
                    rc = lib.axon_start_nrt_profile(ids, len(device_ids))
                else:
                    rc = lib.axon_start_nrt_profile(None, 0)
                if rc != 0:
                    raise RuntimeError(f"axon_start_nrt_profile rc={rc}")
                try:
                    yield
                finally:
                    n = lib.axon_stop_nrt_profile(str(output_dir).encode())
                    print(f"ntff profile: {n} file(s) -> {output_dir}",
                          file=sys.stderr)

            hook = _hook
    except Exception:
        hook = None

    mod = types.ModuleType("antenv.axon_hooks")
    mod.get_axon_ntff_profile_hook = lambda: hook
    mod.set_axon_ntff_profile_hook = lambda h: None
    sys.modules["antenv.axon_hooks"] = mod


def _get_program():
    if "nc" not in _prog_cache:
        _prog_cache["nc"] = _build_program()
    return _prog_cache["nc"]


def kernel(features, enabled):
    feats = np.asarray(features)
    if not bool(np.asarray(enabled)):
        return np.ones(feats.shape, dtype=np.float32)

    _ensure_ntff_hook_module()
    import concourse.bass_utils as _bu
    from concourse.bass_utils import run_bass_kernel_spmd

    # The trace path uploads artifacts to a shared bucket; tolerate
    # sandboxes where that fails.
    if not getattr(_bu, "_upload_patched", False):
        _orig_upload = _bu.upload_artifacts

        def _safe_upload(tmpdir):
            try:
                return _orig_upload(tmpdir)
            except Exception:
                return str(tmpdir)

        _bu.upload_artifacts = _safe_upload
        _bu._upload_patched = True

    nc = _get_program()
    flat = np.ascontiguousarray(feats.reshape(B * C, HW), dtype=np.float32)
    in_maps = [{"x": flat[i * S:(i + 1) * S]} for i in range(N_CORES)]
    res = run_bass_kernel_spmd(nc, in_maps, list(range(N_CORES)))
    _prog_cache["last_res"] = res
    out = np.concatenate([np.asarray(res.results[i]["y"])
                          for i in range(N_CORES)], axis=0)
    return out.reshape(B, C, H, W).astype(np.float32)


# revision 7
# speedup vs baseline: 1.2261x; 1.2261x over previous
"""ChannelBlockImportanceGate kernel for 8 Trainium2 NeuronCores.

Computes, per (b, c) slice of features [8, 256, 132, 132] f32:
  scores = block-sum of |x| over 8x8 blocks (17x17 grid, zero-padded edges)
  top-72 blocks (ties -> lowest index, matching jax.lax.top_k)
  output = per-pixel {0,1} mask upsampled 8x8 (cropped to 132x132)

The straight-through soft term of the reference cancels in the forward
pass (hard - sg(soft) + soft == hard up to ~1ulp), so the output is the
hard mask.

Sharding: purely data parallel. 2048 (b,c) slices -> 256 per core.
Per core: 2 groups of 128 slices; each slice occupies one SBUF
partition so pooling/topk/upsample are per-partition ops with no
cross-partition traffic. Top-72 uses 9 rounds of DVE max8 +
match_replace(-1e30), then mask = (score < 0).

Schedule notes (from perfetto/ntff analysis of earlier versions):
 - Store DMAs need large contiguous SBUF lines: 16KB+ packets run at
   ~26 GB/s/engine, 528B stride-0 broadcast packets only ~9 GB/s
   (39ns/packet overhead). So the mask IS materialized full-size in
   SBUF (in place over the input chunks) before storing.
 - The upsample is the store-gating latency, so it goes through a
   compact row-mask [128, 17*132] (one 132-px row per block-row,
   built by 2 vector copies) followed by ONE 3D broadcast copy per
   block-row, split across the Scalar and GpSimd engines.
 - Group 1's W-pool is split vector/gpsimd so it overlaps group 0's
   topk rounds on vector instead of queueing behind them (the v1
   trace showed a 21us DMA idle gap from exactly this serialization).
 - The last row-chunk is only 12 rows so the final W-pool reduce on
   the critical path (load done -> topk start) is short.
"""

import numpy as np

B, C, H, W = 8, 256, 132, 132
HW = H * W            # 17424
NB = 17               # 8x8 blocks per side (132 padded to 136)
NBLK = NB * NB        # 289
KEEP = 72             # round(289 * 0.25)
N_CORES = 8
S = (B * C) // N_CORES  # 256 slices per core
ROW_CHUNKS = ((0, 40), (40, 80), (80, 120), (120, 132))
NEG = -1.0e30

_prog_cache = {}


def _build_program():
    import concourse.bacc as bacc
    import concourse.mybir as mybir
    import concourse.tile as tile

    f32 = mybir.dt.float32
    X = mybir.AxisListType.X
    ADD = mybir.AluOpType.add

    nc = bacc.Bacc("TRN2", debug=False, num_devices=N_CORES)
    x = nc.dram_tensor("x", (S, HW), f32, kind="ExternalInput")
    y = nc.dram_tensor("y", (S, HW), f32, kind="ExternalOutput")

    with tile.TileContext(nc) as tc:
        with (
            tc.tile_pool(name="big", bufs=2) as bigp,
            tc.tile_pool(name="med", bufs=2) as medp,
            tc.tile_pool(name="small", bufs=2) as smallp,
        ):
            for g in range(S // 128):
                p0 = g * 128
                chunks = []
                for k, (r0, r1) in enumerate(ROW_CHUNKS):
                    ch = bigp.tile([128, (r1 - r0) * W], f32,
                                   name=f"ch_g{g}k{k}", tag=f"chunk{k}")
                    nc.sync.dma_start(out=ch[:, :],
                                      in_=x[p0:p0 + 128, r0 * W:r1 * W])
                    chunks.append(ch)

                # W-pool: per image row, |x| summed over 8-col groups
                # (16 full groups + one 4-col partial group).
                wsum = medp.tile([128, H * NB], f32,
                                 name=f"wsum_g{g}", tag="wsum")
                ws3 = wsum.rearrange("p (r t) -> p r t", t=NB)
                if g > 0:
                    # Ordering token: the v1 trace showed the scheduler
                    # interleaving this group's 5us W-pool reduces between
                    # the previous group's topk rounds, delaying its mask
                    # (and thus all stores) by ~20us. A 1-element copy that
                    # READS the previous group's mask and WRITES into every
                    # row-region of this group's wsum creates RAW+WAW deps
                    # that pin all W-pool reduces after the previous topk.
                    nc.vector.tensor_copy(
                        out=ws3[0:1, :, 0:1],
                        in_=prev_mask[0:1, 0:1].unsqueeze(2)
                        .broadcast_to((1, H, 1)))
                for k, (r0, r1) in enumerate(ROW_CHUNKS):
                    v = chunks[k].rearrange("p (r w) -> p r w", w=W)
                    nc.vector.tensor_reduce(
                        out=ws3[:, r0:r1, 0:16],
                        in_=v[:, :, 0:128].rearrange("p r (q c) -> p r q c", c=8),
                        axis=X, op=ADD, apply_absolute_value=True)
                    nc.vector.tensor_reduce(
                        out=ws3[:, r0:r1, 16:17],
                        in_=v[:, :, 128:132],
                        axis=X, op=ADD, apply_absolute_value=True)

                # H-pool: row sums summed over 8-row groups (16 full + 4-row
                # partial) -> scores [128, 289], layout h*17 + w.
                scores = smallp.tile([128, NBLK], f32,
                                     name=f"scores_g{g}", tag="scores")
                sc3 = scores.rearrange("p (h t) -> p h t", t=NB)
                nc.vector.tensor_reduce(
                    out=sc3[:, 0:16, :],
                    in_=ws3[:, 0:128, :].rearrange("p (h r) t -> p h t r", r=8),
                    axis=X, op=ADD)
                nc.vector.tensor_reduce(
                    out=sc3[:, 16:17, :],
                    in_=ws3[:, 128:132, :].rearrange("p r t -> p t r"),
                    axis=X, op=ADD)

                # Top-72 per partition: 9 rounds of max8 + match_replace.
                # match_replace replaces the first unmatched occurrence, so
                # ties resolve to the lowest index like jax.lax.top_k.
                for it in range(KEEP // 8):
                    m8 = smallp.tile([128, 8], f32,
                                     name=f"m8_g{g}i{it}", tag="m8")
                    nc.vector.max(out=m8[:, :], in_=scores[:, :])
                    nc.vector.match_replace(out=scores[:, :],
                                            in_to_replace=m8[:, :],
                                            in_values=scores[:, :],
                                            imm_value=NEG)

                # Block mask: replaced entries are -1e30, real scores are >= 0.
                mask = smallp.tile([128, NBLK], f32,
                                   name=f"mask_g{g}", tag="mask")
                nc.vector.tensor_scalar(out=mask[:, :], in0=scores[:, :],
                                        scalar1=0.0, scalar2=None,
                                        op0=mybir.AluOpType.is_lt)
                m3 = mask.rearrange("p (h t) -> p h t", t=NB)
                prev_mask = mask

                # Row-mask [p, h, w]: one 132-px row per block-row (the 8x
                # horizontal upsample), built by two small vector copies.
                rm = medp.tile([128, NB * W], f32, name=f"rm_g{g}", tag="rm")
                rm3 = rm.rearrange("p (h w) -> p h w", w=W)
                nc.vector.tensor_copy(
                    out=rm3[:, :, 0:128].rearrange("p h (q c) -> p h q c", c=8),
                    in_=(m3[:, :, 0:16].unsqueeze(3)
                         .broadcast_to((128, NB, 16, 8))))
                nc.vector.tensor_copy(
                    out=rm3[:, :, 128:132],
                    in_=m3[:, :, 16:17].broadcast_to((128, NB, 4)))

                # Vertical 8x upsample in place over the feature chunks:
                # one 3D broadcast copy per block-row, alternating between
                # the Scalar and GpSimd engines, then store the chunk.
                flip = 0
                for k, (r0, r1) in enumerate(ROW_CHUNKS):
                    v = chunks[k].rearrange("p (r w) -> p r w", w=W)
                    for h in range(r0 // 8, (r1 + 7) // 8):
                        a = h * 8 - r0
                        nr = min(8, H - h * 8)
                        src = rm3[:, h:h + 1, :].broadcast_to((128, nr, W))
                        if flip == 0:
                            nc.scalar.copy(out=v[:, a:a + nr, :], in_=src)
                        else:
                            nc.gpsimd.tensor_copy(out=v[:, a:a + nr, :],
                                                  in_=src)
                        flip ^= 1
                    nc.sync.dma_start(out=y[p0:p0 + 128, r0 * W:r1 * W],
                                      in_=chunks[k][:, :])
    nc.compile()
    return nc


def _ensure_ntff_hook_module():
    """bass_utils' trace path does `from antenv.axon_hooks import
    get_axon_ntff_profile_hook` — a module this image doesn't ship.
    Register an equivalent (ctypes into libaxon_pjrt.so, mirroring
    trn_boot._ntff_profile_via_ctypes) so BASS_TRACE=1 works; degrade
    to a None hook (trace skipped) when unavailable."""
    import sys
    import types

    try:
        import antenv.axon_hooks  # noqa: F401
        return
    except Exception:
        pass

    hook = None
    try:
        import contextlib
        import ctypes

        so_path = "/opt/axon/libaxon_pjrt.so"
        lib = ctypes.CDLL(so_path)
        if hasattr(lib, "axon_start_nrt_profile"):
            lib.axon_start_nrt_profile.argtypes = [
                ctypes.POINTER(ctypes.c_int64), ctypes.c_size_t]
            lib.axon_start_nrt_profile.restype = ctypes.c_int64
            lib.axon_stop_nrt_profile.argtypes = [ctypes.c_char_p]
            lib.axon_stop_nrt_profile.restype = ctypes.c_int64

            @contextlib.contextmanager
            def _hook(output_dir, device_ids):
                import jax
                jax.devices()
                if device_ids:
                    ids = (ctypes.c_int64 * len(device_ids))(*device_ids)
                    rc = lib.axon_start_nrt_profile(ids, len(device_ids))
                else:
                    rc = lib.axon_start_nrt_profile(None, 0)
                if rc != 0:
                    raise RuntimeError(f"axon_start_nrt_profile rc={rc}")
                try:
                    yield
                finally:
                    n = lib.axon_stop_nrt_profile(str(output_dir).encode())
                    print(f"ntff profile: {n} file(s) -> {output_dir}",
                          file=sys.stderr)

            hook = _hook
    except Exception:
        hook = None

    mod = types.ModuleType("antenv.axon_hooks")
    mod.get_axon_ntff_profile_hook = lambda: hook
    mod.set_axon_ntff_profile_hook = lambda h: None
    sys.modules["antenv.axon_hooks"] = mod


def _get_program():
    if "nc" not in _prog_cache:
        _prog_cache["nc"] = _build_program()
    return _prog_cache["nc"]


def kernel(features, enabled):
    feats = np.asarray(features)
    if not bool(np.asarray(enabled)):
        return np.ones(feats.shape, dtype=np.float32)

    _ensure_ntff_hook_module()
    import concourse.bass_utils as _bu
    from concourse.bass_utils import run_bass_kernel_spmd

    # The trace path uploads artifacts to a shared bucket; tolerate
    # sandboxes where that fails.
    if not getattr(_bu, "_upload_patched", False):
        _orig_upload = _bu.upload_artifacts

        def _safe_upload(tmpdir):
            try:
                return _orig_upload(tmpdir)
            except Exception:
                return str(tmpdir)

        _bu.upload_artifacts = _safe_upload
        _bu._upload_patched = True

    nc = _get_program()
    flat = np.ascontiguousarray(feats.reshape(B * C, HW), dtype=np.float32)
    in_maps = [{"x": flat[i * S:(i + 1) * S]} for i in range(N_CORES)]
    res = run_bass_kernel_spmd(nc, in_maps, list(range(N_CORES)))
    _prog_cache["last_res"] = res
    out = np.concatenate([np.asarray(res.results[i]["y"])
                          for i in range(N_CORES)], axis=0)
    return out.reshape(B, C, H, W).astype(np.float32)


# revision 8
# speedup vs baseline: 1.5242x; 1.2431x over previous
"""ChannelBlockImportanceGate kernel for 8 Trainium2 NeuronCores.

Computes, per (b, c) slice of features [8, 256, 132, 132] f32:
  scores = block-sum of |x| over 8x8 blocks (17x17 grid, zero-padded edges)
  top-72 blocks (ties -> lowest index, matching jax.lax.top_k)
  output = per-pixel {0,1} mask upsampled 8x8 (cropped to 132x132)

The straight-through soft term of the reference cancels in the forward
pass (hard - sg(soft) + soft == hard up to ~1ulp), so the output is the
hard mask.

Sharding: purely data parallel. 2048 (b,c) slices -> 256 per core.
Per core: 2 groups of 128 slices; each slice occupies one SBUF
partition so pooling/topk/upsample are per-partition ops with no
cross-partition traffic. Top-72 uses 9 rounds of DVE max8 +
match_replace(-1e30), then mask = (score < 0).

Schedule notes (from perfetto/ntff analysis of earlier versions):
 - Store DMAs need large contiguous SBUF lines: 16KB+ packets run at
   ~26 GB/s/engine, 528B stride-0 broadcast packets only ~9 GB/s
   (39ns/packet overhead). So the mask IS materialized full-size in
   SBUF (in place over the input chunks) before storing.
 - The upsample is the store-gating latency, so it goes through a
   compact row-mask [128, 17*132] (one 132-px row per block-row,
   built by 2 vector copies) followed by ONE 3D broadcast copy per
   block-row, split across the Scalar and GpSimd engines.
 - Group 1's W-pool is split vector/gpsimd so it overlaps group 0's
   topk rounds on vector instead of queueing behind them (the v1
   trace showed a 21us DMA idle gap from exactly this serialization).
 - The last row-chunk is only 12 rows so the final W-pool reduce on
   the critical path (load done -> topk start) is short.
"""

import numpy as np

B, C, H, W = 8, 256, 132, 132
HW = H * W            # 17424
NB = 17               # 8x8 blocks per side (132 padded to 136)
NBLK = NB * NB        # 289
KEEP = 72             # round(289 * 0.25)
N_CORES = 8
S = (B * C) // N_CORES  # 256 slices per core
ROW_CHUNKS = ((0, 40), (40, 80), (80, 120), (120, 132))
NEG = -1.0e30

_prog_cache = {}


def _build_program():
    import concourse.bacc as bacc
    import concourse.mybir as mybir
    import concourse.tile as tile

    f32 = mybir.dt.float32
    X = mybir.AxisListType.X
    ADD = mybir.AluOpType.add

    nc = bacc.Bacc("TRN2", debug=False, num_devices=N_CORES)
    x = nc.dram_tensor("x", (S, HW), f32, kind="ExternalInput")
    y = nc.dram_tensor("y", (S, HW), f32, kind="ExternalOutput")

    with tile.TileContext(nc) as tc:
        with (
            tc.tile_pool(name="big", bufs=2) as bigp,
            tc.tile_pool(name="med", bufs=2) as medp,
            tc.tile_pool(name="small", bufs=2) as smallp,
        ):
            # All load DMAs are emitted (and thus queued) before any
            # store DMA so a store trigger waiting on its upsample
            # semaphore can never head-of-line-block a load trigger on
            # the shared sync queue (cost v3 ~22us on group 1's last
            # chunk). Tile WAR semaphores still pace group 1's loads
            # behind group 0's W-pool reads of the shared buffers.
            all_chunks = []
            for g in range(S // 128):
                p0 = g * 128
                chunks = []
                for k, (r0, r1) in enumerate(ROW_CHUNKS):
                    ch = bigp.tile([128, (r1 - r0) * W], f32,
                                   name=f"ch_g{g}k{k}", tag=f"chunk{k}")
                    nc.sync.dma_start(out=ch[:, :],
                                      in_=x[p0:p0 + 128, r0 * W:r1 * W])
                    chunks.append(ch)
                all_chunks.append(chunks)

            for g in range(S // 128):
                p0 = g * 128
                chunks = all_chunks[g]

                # W-pool: per image row, |x| summed over 8-col groups
                # (16 full groups + one 4-col partial group).
                wsum = medp.tile([128, H * NB], f32,
                                 name=f"wsum_g{g}", tag="wsum")
                ws3 = wsum.rearrange("p (r t) -> p r t", t=NB)
                if g > 0:
                    # Ordering token: the v1 trace showed the scheduler
                    # interleaving this group's 5us W-pool reduces between
                    # the previous group's topk rounds, delaying its mask
                    # (and thus all stores) by ~20us. A 1-element copy that
                    # READS the previous group's mask and WRITES into every
                    # row-region of this group's wsum creates RAW+WAW deps
                    # that pin all W-pool reduces after the previous topk.
                    nc.vector.tensor_copy(
                        out=ws3[0:1, :, 0:1],
                        in_=prev_mask[0:1, 0:1].unsqueeze(2)
                        .broadcast_to((1, H, 1)))
                for k, (r0, r1) in enumerate(ROW_CHUNKS):
                    v = chunks[k].rearrange("p (r w) -> p r w", w=W)
                    nc.vector.tensor_reduce(
                        out=ws3[:, r0:r1, 0:16],
                        in_=v[:, :, 0:128].rearrange("p r (q c) -> p r q c", c=8),
                        axis=X, op=ADD, apply_absolute_value=True)
                    nc.vector.tensor_reduce(
                        out=ws3[:, r0:r1, 16:17],
                        in_=v[:, :, 128:132],
                        axis=X, op=ADD, apply_absolute_value=True)

                # H-pool: row sums summed over 8-row groups (16 full + 4-row
                # partial) -> scores [128, 289], layout h*17 + w.
                scores = smallp.tile([128, NBLK], f32,
                                     name=f"scores_g{g}", tag="scores")
                sc3 = scores.rearrange("p (h t) -> p h t", t=NB)
                nc.vector.tensor_reduce(
                    out=sc3[:, 0:16, :],
                    in_=ws3[:, 0:128, :].rearrange("p (h r) t -> p h t r", r=8),
                    axis=X, op=ADD)
                nc.vector.tensor_reduce(
                    out=sc3[:, 16:17, :],
                    in_=ws3[:, 128:132, :].rearrange("p r t -> p t r"),
                    axis=X, op=ADD)

                # Top-72 per partition: 9 rounds of max8 + match_replace.
                # match_replace replaces the first unmatched occurrence, so
                # ties resolve to the lowest index like jax.lax.top_k.
                for it in range(KEEP // 8):
                    m8 = smallp.tile([128, 8], f32,
                                     name=f"m8_g{g}i{it}", tag="m8")
                    nc.vector.max(out=m8[:, :], in_=scores[:, :])
                    nc.vector.match_replace(out=scores[:, :],
                                            in_to_replace=m8[:, :],
                                            in_values=scores[:, :],
                                            imm_value=NEG)

                # Block mask: replaced entries are -1e30, real scores are >= 0.
                mask = smallp.tile([128, NBLK], f32,
                                   name=f"mask_g{g}", tag="mask")
                nc.vector.tensor_scalar(out=mask[:, :], in0=scores[:, :],
                                        scalar1=0.0, scalar2=None,
                                        op0=mybir.AluOpType.is_lt)
                m3 = mask.rearrange("p (h t) -> p h t", t=NB)
                prev_mask = mask

                # Row-mask [p, h, w]: one 132-px row per block-row (the 8x
                # horizontal upsample), built by two small vector copies.
                rm = medp.tile([128, NB * W], f32, name=f"rm_g{g}", tag="rm")
                rm3 = rm.rearrange("p (h w) -> p h w", w=W)
                nc.vector.tensor_copy(
                    out=rm3[:, :, 0:128].rearrange("p h (q c) -> p h q c", c=8),
                    in_=(m3[:, :, 0:16].unsqueeze(3)
                         .broadcast_to((128, NB, 16, 8))))
                nc.vector.tensor_copy(
                    out=rm3[:, :, 128:132],
                    in_=m3[:, :, 16:17].broadcast_to((128, NB, 4)))

                # Vertical 8x upsample in place over the feature chunks:
                # one 3D broadcast copy per block-row, then store the chunk.
                # GpSimd copies measure 3.69us vs scalar's 1.16us, so gpsimd
                # only takes ONE copy per full chunk (balanced ~3:1) and
                # none on the small last chunk, keeping it off the tail.
                for k, (r0, r1) in enumerate(ROW_CHUNKS):
                    v = chunks[k].rearrange("p (r w) -> p r w", w=W)
                    hgs = list(range(r0 // 8, (r1 + 7) // 8))
                    for i, h in enumerate(hgs):
                        a = h * 8 - r0
                        nr = min(8, H - h * 8)
                        src = rm3[:, h:h + 1, :].broadcast_to((128, nr, W))
                        if i == 0 and len(hgs) > 2:
                            nc.gpsimd.tensor_copy(out=v[:, a:a + nr, :],
                                                  in_=src)
                        else:
                            nc.scalar.copy(out=v[:, a:a + nr, :], in_=src)
                    nc.sync.dma_start(out=y[p0:p0 + 128, r0 * W:r1 * W],
                                      in_=chunks[k][:, :])
    nc.compile()
    return nc


def _ensure_ntff_hook_module():
    """bass_utils' trace path does `from antenv.axon_hooks import
    get_axon_ntff_profile_hook` — a module this image doesn't ship.
    Register an equivalent (ctypes into libaxon_pjrt.so, mirroring
    trn_boot._ntff_profile_via_ctypes) so BASS_TRACE=1 works; degrade
    to a None hook (trace skipped) when unavailable."""
    import sys
    import types

    try:
        import antenv.axon_hooks  # noqa: F401
        return
    except Exception:
        pass

    hook = None
    try:
        import contextlib
        import ctypes

        so_path = "/opt/axon/libaxon_pjrt.so"
        lib = ctypes.CDLL(so_path)
        if hasattr(lib, "axon_start_nrt_profile"):
            lib.axon_start_nrt_profile.argtypes = [
                ctypes.POINTER(ctypes.c_int64), ctypes.c_size_t]
            lib.axon_start_nrt_profile.restype = ctypes.c_int64
            lib.axon_stop_nrt_profile.argtypes = [ctypes.c_char_p]
            lib.axon_stop_nrt_profile.restype = ctypes.c_int64

            @contextlib.contextmanager
            def _hook(output_dir, device_ids):
                import jax
                jax.devices()
                if device_ids:
                    ids = (ctypes.c_int64 * len(device_ids))(*device_ids)
                    rc = lib.axon_start_nrt_profile(ids, len(device_ids))
                else:
                    rc = lib.axon_start_nrt_profile(None, 0)
                if rc != 0:
                    raise RuntimeError(f"axon_start_nrt_profile rc={rc}")
                try:
                    yield
                finally:
                    n = lib.axon_stop_nrt_profile(str(output_dir).encode())
                    print(f"ntff profile: {n} file(s) -> {output_dir}",
                          file=sys.stderr)

            hook = _hook
    except Exception:
        hook = None

    mod = types.ModuleType("antenv.axon_hooks")
    mod.get_axon_ntff_profile_hook = lambda: hook
    mod.set_axon_ntff_profile_hook = lambda h: None
    sys.modules["antenv.axon_hooks"] = mod


def _get_program():
    if "nc" not in _prog_cache:
        _prog_cache["nc"] = _build_program()
    return _prog_cache["nc"]


def kernel(features, enabled):
    feats = np.asarray(features)
    if not bool(np.asarray(enabled)):
        return np.ones(feats.shape, dtype=np.float32)

    _ensure_ntff_hook_module()
    import concourse.bass_utils as _bu
    from concourse.bass_utils import run_bass_kernel_spmd

    # The trace path uploads artifacts to a shared bucket; tolerate
    # sandboxes where that fails.
    if not getattr(_bu, "_upload_patched", False):
        _orig_upload = _bu.upload_artifacts

        def _safe_upload(tmpdir):
            try:
                return _orig_upload(tmpdir)
            except Exception:
                return str(tmpdir)

        _bu.upload_artifacts = _safe_upload
        _bu._upload_patched = True

    nc = _get_program()
    flat = np.ascontiguousarray(feats.reshape(B * C, HW), dtype=np.float32)
    in_maps = [{"x": flat[i * S:(i + 1) * S]} for i in range(N_CORES)]
    res = run_bass_kernel_spmd(nc, in_maps, list(range(N_CORES)))
    _prog_cache["last_res"] = res
    out = np.concatenate([np.asarray(res.results[i]["y"])
                          for i in range(N_CORES)], axis=0)
    return out.reshape(B, C, H, W).astype(np.float32)


# revision 9
# speedup vs baseline: 1.5700x; 1.0301x over previous
"""ChannelBlockImportanceGate kernel for 8 Trainium2 NeuronCores.

Computes, per (b, c) slice of features [8, 256, 132, 132] f32:
  scores = block-sum of |x| over 8x8 blocks (17x17 grid, zero-padded edges)
  top-72 blocks (ties -> lowest index, matching jax.lax.top_k)
  output = per-pixel {0,1} mask upsampled 8x8 (cropped to 132x132)

The straight-through soft term of the reference cancels in the forward
pass (hard - sg(soft) + soft == hard up to ~1ulp), so the output is the
hard mask.

Sharding: purely data parallel. 2048 (b,c) slices -> 256 per core.
Per core: 2 groups of 128 slices; each slice occupies one SBUF
partition so pooling/topk/upsample are per-partition ops with no
cross-partition traffic. Top-72 uses 9 rounds of DVE max8 +
match_replace(-1e30), then mask = (score < 0).

Schedule notes (from perfetto/ntff analysis of earlier versions):
 - Store DMAs need large contiguous SBUF lines: 16KB+ packets run at
   ~26 GB/s/engine, 528B stride-0 broadcast packets only ~9 GB/s
   (39ns/packet overhead). So the mask IS materialized full-size in
   SBUF (in place over the input chunks) before storing.
 - The upsample is the store-gating latency, so it goes through a
   compact row-mask [128, 17*132] (one 132-px row per block-row,
   built by 2 vector copies) followed by ONE 3D broadcast copy per
   block-row, split across the Scalar and GpSimd engines.
 - Group 1's W-pool is split vector/gpsimd so it overlaps group 0's
   topk rounds on vector instead of queueing behind them (the v1
   trace showed a 21us DMA idle gap from exactly this serialization).
 - The last row-chunk is only 12 rows so the final W-pool reduce on
   the critical path (load done -> topk start) is short.
"""

import numpy as np

B, C, H, W = 8, 256, 132, 132
HW = H * W            # 17424
NB = 17               # 8x8 blocks per side (132 padded to 136)
NBLK = NB * NB        # 289
KEEP = 72             # round(289 * 0.25)
N_CORES = 8
S = (B * C) // N_CORES  # 256 slices per core
ROW_CHUNKS = ((0, 32), (32, 56), (56, 80), (80, 104), (104, 120),
              (120, 132))
NEG = -1.0e30

_prog_cache = {}


def _build_program():
    import concourse.bacc as bacc
    import concourse.mybir as mybir
    import concourse.tile as tile

    f32 = mybir.dt.float32
    X = mybir.AxisListType.X
    ADD = mybir.AluOpType.add

    nc = bacc.Bacc("TRN2", debug=False, num_devices=N_CORES)
    x = nc.dram_tensor("x", (S, HW), f32, kind="ExternalInput")
    y = nc.dram_tensor("y", (S, HW), f32, kind="ExternalOutput")

    with tile.TileContext(nc) as tc:
        with (
            tc.tile_pool(name="big", bufs=2) as bigp,
            tc.tile_pool(name="med", bufs=2) as medp,
            tc.tile_pool(name="small", bufs=2) as smallp,
        ):
            # All load DMAs are emitted (and thus queued) before any
            # store DMA so a store trigger waiting on its upsample
            # semaphore can never head-of-line-block a load trigger on
            # the shared sync queue (cost v3 ~22us on group 1's last
            # chunk). Tile WAR semaphores still pace group 1's loads
            # behind group 0's W-pool reads of the shared buffers.
            all_chunks = []
            for g in range(S // 128):
                p0 = g * 128
                chunks = []
                for k, (r0, r1) in enumerate(ROW_CHUNKS):
                    ch = bigp.tile([128, (r1 - r0) * W], f32,
                                   name=f"ch_g{g}k{k}", tag=f"chunk{k}")
                    nc.sync.dma_start(out=ch[:, :],
                                      in_=x[p0:p0 + 128, r0 * W:r1 * W])
                    chunks.append(ch)
                all_chunks.append(chunks)

            for g in range(S // 128):
                p0 = g * 128
                chunks = all_chunks[g]

                # W-pool: per image row, |x| summed over 8-col groups
                # (16 full groups + one 4-col partial group). Output is
                # T-MAJOR [p, t(17), r(136)] so the H-pool's reduce axis
                # (rows within a block-row) is contiguous: strided-X
                # reduces measure 1.54ns/elem vs 1.07 contiguous.
                wsum = medp.tile([128, NB * 136], f32,
                                 name=f"wsum_g{g}", tag="wsum")
                wsT3 = wsum.rearrange("p (t r) -> p t r", r=136)
                if g > 0:
                    # Ordering token: the v1 trace showed the scheduler
                    # interleaving this group's 5us W-pool reduces between
                    # the previous group's topk rounds, delaying its mask
                    # (and thus all stores) by ~20us. A 1-element copy that
                    # READS the previous group's mask and WRITES into every
                    # row-region of this group's wsum creates RAW+WAW deps
                    # that pin all W-pool reduces after the previous topk.
                    nc.vector.tensor_copy(
                        out=wsum[0:1, 0:136],
                        in_=prev_mask[0:1, 0:1].broadcast_to((1, 136)))
                for k, (r0, r1) in enumerate(ROW_CHUNKS):
                    v = chunks[k].rearrange("p (r w) -> p r w", w=W)
                    nc.vector.tensor_reduce(
                        out=(wsT3[:, 0:16, r0:r1]
                             .rearrange("p t r -> p r t")),
                        in_=v[:, :, 0:128].rearrange("p r (q c) -> p r q c", c=8),
                        axis=X, op=ADD, apply_absolute_value=True)
                    nc.vector.tensor_reduce(
                        out=(wsT3[:, 16:17, r0:r1]
                             .rearrange("p t r -> p r t")),
                        in_=v[:, :, 128:132],
                        axis=X, op=ADD, apply_absolute_value=True)

                # H-pool: row sums summed over 8-row groups (16 full + 4-row
                # partial) -> scores [128, 289], T-MAJOR layout t*17 + h,
                # reduce axis contiguous in wsum.
                scores = smallp.tile([128, NBLK], f32,
                                     name=f"scores_g{g}", tag="scores")
                scT3 = scores.rearrange("p (t h) -> p t h", h=NB)
                nc.vector.tensor_reduce(
                    out=scT3[:, :, 0:16],
                    in_=wsT3[:, :, 0:128].rearrange("p t (h r) -> p t h r", r=8),
                    axis=X, op=ADD)
                nc.vector.tensor_reduce(
                    out=scT3[:, :, 16:17],
                    in_=wsT3[:, :, 128:132].unsqueeze(2),
                    axis=X, op=ADD)

                # Top-72 per partition: 9 rounds of max8 + match_replace.
                # match_replace replaces the first unmatched occurrence, so
                # ties resolve to the lowest index like jax.lax.top_k.
                for it in range(KEEP // 8):
                    m8 = smallp.tile([128, 8], f32,
                                     name=f"m8_g{g}i{it}", tag="m8")
                    nc.vector.max(out=m8[:, :], in_=scores[:, :])
                    nc.vector.match_replace(out=scores[:, :],
                                            in_to_replace=m8[:, :],
                                            in_values=scores[:, :],
                                            imm_value=NEG)

                # Block mask: replaced entries are -1e30, real scores are >= 0.
                mask = smallp.tile([128, NBLK], f32,
                                   name=f"mask_g{g}", tag="mask")
                nc.vector.tensor_scalar(out=mask[:, :], in0=scores[:, :],
                                        scalar1=0.0, scalar2=None,
                                        op0=mybir.AluOpType.is_lt)
                # mask layout is t-major: mask[p, t*17 + h]; mT views it
                # h-major for the row-mask build.
                mT = mask.rearrange("p (t h) -> p h t", h=NB)
                prev_mask = mask

                # Row-mask [p, h, w]: one 132-px row per block-row (the 8x
                # horizontal upsample), built by two small vector copies.
                rm = medp.tile([128, NB * W], f32, name=f"rm_g{g}", tag="rm")
                rm3 = rm.rearrange("p (h w) -> p h w", w=W)
                nc.vector.tensor_copy(
                    out=rm3[:, :, 0:128].rearrange("p h (q c) -> p h q c", c=8),
                    in_=(mT[:, :, 0:16].unsqueeze(3)
                         .broadcast_to((128, NB, 16, 8))))
                nc.vector.tensor_copy(
                    out=rm3[:, :, 128:132],
                    in_=mT[:, :, 16:17].broadcast_to((128, NB, 4)))

                # Vertical 8x upsample in place over the feature chunks:
                # one 3D broadcast copy per block-row, then store the chunk.
                # GpSimd copies measure 3.69us vs scalar's 1.16us, so gpsimd
                # only takes ONE copy per full chunk (balanced ~3:1) and
                # none on the small last chunk, keeping it off the tail.
                for k, (r0, r1) in enumerate(ROW_CHUNKS):
                    v = chunks[k].rearrange("p (r w) -> p r w", w=W)
                    hgs = list(range(r0 // 8, (r1 + 7) // 8))
                    for i, h in enumerate(hgs):
                        a = h * 8 - r0
                        nr = min(8, H - h * 8)
                        src = rm3[:, h:h + 1, :].broadcast_to((128, nr, W))
                        if i == 0 and len(hgs) > 2:
                            nc.gpsimd.tensor_copy(out=v[:, a:a + nr, :],
                                                  in_=src)
                        else:
                            nc.scalar.copy(out=v[:, a:a + nr, :], in_=src)
                    nc.sync.dma_start(out=y[p0:p0 + 128, r0 * W:r1 * W],
                                      in_=chunks[k][:, :])
    nc.compile()
    return nc


def _ensure_ntff_hook_module():
    """bass_utils' trace path does `from antenv.axon_hooks import
    get_axon_ntff_profile_hook` — a module this image doesn't ship.
    Register an equivalent (ctypes into libaxon_pjrt.so, mirroring
    trn_boot._ntff_profile_via_ctypes) so BASS_TRACE=1 works; degrade
    to a None hook (trace skipped) when unavailable."""
    import sys
    import types

    try:
        import antenv.axon_hooks  # noqa: F401
        return
    except Exception:
        pass

    hook = None
    try:
        import contextlib
        import ctypes

        so_path = "/opt/axon/libaxon_pjrt.so"
        lib = ctypes.CDLL(so_path)
        if hasattr(lib, "axon_start_nrt_profile"):
            lib.axon_start_nrt_profile.argtypes = [
                ctypes.POINTER(ctypes.c_int64), ctypes.c_size_t]
            lib.axon_start_nrt_profile.restype = ctypes.c_int64
            lib.axon_stop_nrt_profile.argtypes = [ctypes.c_char_p]
            lib.axon_stop_nrt_profile.restype = ctypes.c_int64

            @contextlib.contextmanager
            def _hook(output_dir, device_ids):
                import jax
                jax.devices()
                if device_ids:
                    ids = (ctypes.c_int64 * len(device_ids))(*device_ids)
                    rc = lib.axon_start_nrt_profile(ids, len(device_ids))
                else:
                    rc = lib.axon_start_nrt_profile(None, 0)
                if rc != 0:
                    raise RuntimeError(f"axon_start_nrt_profile rc={rc}")
                try:
                    yield
                finally:
                    n = lib.axon_stop_nrt_profile(str(output_dir).encode())
                    print(f"ntff profile: {n} file(s) -> {output_dir}",
                          file=sys.stderr)

            hook = _hook
    except Exception:
        hook = None

    mod = types.ModuleType("antenv.axon_hooks")
    mod.get_axon_ntff_profile_hook = lambda: hook
    mod.set_axon_ntff_profile_hook = lambda h: None
    sys.modules["antenv.axon_hooks"] = mod


def _get_program():
    if "nc" not in _prog_cache:
        _prog_cache["nc"] = _build_program()
    return _prog_cache["nc"]


def kernel(features, enabled):
    feats = np.asarray(features)
    if not bool(np.asarray(enabled)):
        return np.ones(feats.shape, dtype=np.float32)

    _ensure_ntff_hook_module()
    import concourse.bass_utils as _bu
    from concourse.bass_utils import run_bass_kernel_spmd

    # The trace path uploads artifacts to a shared bucket; tolerate
    # sandboxes where that fails.
    if not getattr(_bu, "_upload_patched", False):
        _orig_upload = _bu.upload_artifacts

        def _safe_upload(tmpdir):
            try:
                return _orig_upload(tmpdir)
            except Exception:
                return str(tmpdir)

        _bu.upload_artifacts = _safe_upload
        _bu._upload_patched = True

    nc = _get_program()
    flat = np.ascontiguousarray(feats.reshape(B * C, HW), dtype=np.float32)
    in_maps = [{"x": flat[i * S:(i + 1) * S]} for i in range(N_CORES)]
    res = run_bass_kernel_spmd(nc, in_maps, list(range(N_CORES)))
    _prog_cache["last_res"] = res
    out = np.concatenate([np.asarray(res.results[i]["y"])
                          for i in range(N_CORES)], axis=0)
    return out.reshape(B, C, H, W).astype(np.float32)
